# revision 6
# baseline (speedup 1.0000x reference)
"""Paged KV-cache decode attention with ALiBi (Baichuan-style), fused
QKV + attention + output projection, tensor-parallel over heads across
8 Trainium2 NeuronCores.

bf16 design (memory-regime: per-core HBM traffic ~41 MB bf16 vs 84 MB
fp32; per-core roofline ~113 us @ 358 GB/s):
  - All large tensors (qkv_weight, o_proj_weight, K/V cache) cast to
    bf16 on host and staged in on-chip-friendly layouts so every device
    DMA is >=0.6-2.6 MB with >=2.5 KB contiguous runs per partition.
  - q/k/v computed weight-stationary (bf16 FWL makes the 128x128 weight
    loads cheap) directly in transposed orientation qT/kT/vT [128(d),
    5(h)*4(b)].
  - New-token K/V handled WITHOUT cache scatters: the host bakes
    bias[pos] = -1e30 (so the stale cache column at pos contributes 0),
    and the new token's attention term a_pos = exp(q . k_new) is
    computed for all 20 (b,h) at once via an elementwise mul + ones
    matmul; its rank-1 contribution a_pos * v_new is added before
    normalization.
  - softmax without max-subtraction (scores are O(5); exp safe in
    fp32), masking baked into the host-precomputed additive fp32 bias.
  - o_proj computed in natural orientation (out [4, 5120] per core,
    attn stationary, weights moving, N=512 matmuls); host sums the 8
    partial products (the "all-reduce").
"""

import math
import os
import sys
from contextlib import ExitStack

import numpy as np
import ml_dtypes

sys.path.insert(0, "/opt/trn_rl_repo")

BF16 = ml_dtypes.bfloat16

B = 4
E = 5120
H = 40
D = 128
BS = 16
NB = 512
MB = 128
S = MB * BS  # 2048
NCORES = 8
HPC = H // NCORES   # 5 heads per core
EPC = HPC * D       # 640
NCH = S // 128      # 16 chunks of 128 tokens

NEG = -1.0e30


def _alibi_slopes(num_heads):
    cp2 = 2 ** int(math.floor(math.log2(num_heads)))
    base = 2.0 ** (-(2.0 ** (-(math.log2(cp2) - 3))))
    slopes = base ** np.arange(1, cp2 + 1, dtype=np.float64)
    if cp2 != num_heads:
        extra_base = 2.0 ** (-(2.0 ** (-(math.log2(2 * cp2) - 3))))
        n_rem = min(cp2, num_heads - cp2)
        extra = extra_base ** np.arange(1, 1 + 2 * n_rem, 2, dtype=np.float64)
        slopes = np.concatenate([slopes, extra])
    return slopes.astype(np.float32)


_PROGRAM_CACHE = {}
LAST_RESULTS = None  # BassKernelResults of the most recent run (for test.py)


def _build_program(nch):
    """Build the SPMD Bass program. nch (per-sequence chunk counts) is
    baked statically; all other seq-length dependence lives in host data."""
    import concourse.bacc as bacc
    import concourse.bass as bass
    import concourse.tile as tile
    from concourse import mybir

    f32 = mybir.dt.float32
    bf16 = mybir.dt.bfloat16
    nc = bacc.Bacc()

    hT = nc.declare_dram_parameter("hT", [128, 40 * B], bf16, isOutput=False)
    qkvw = nc.declare_dram_parameter("qkvw", [3, 5, 128, 8 * EPC], bf16, isOutput=False)
    ow = nc.declare_dram_parameter("ow", [10, 128, HPC * 512], bf16, isOutput=False)
    kt = nc.declare_dram_parameter("kt", [B, 128, HPC, S], bf16, isOutput=False)
    vt = nc.declare_dram_parameter("vt", [B, 128, HPC, NCH, D], bf16, isOutput=False)
    bias = nc.declare_dram_parameter("bias", [128, B * HPC * NCH], f32, isOutput=False)
    out = nc.declare_dram_parameter("out", [B, E], f32, isOutput=True)

    nmax = max(nch)

    with tile.TileContext(nc) as tc, ExitStack() as ctx:
        consts = ctx.enter_context(tc.tile_pool(name="consts", bufs=1))
        wpool = ctx.enter_context(tc.tile_pool(name="wpool", bufs=3))
        kpool = ctx.enter_context(tc.tile_pool(name="kpool", bufs=2))
        vpool = ctx.enter_context(tc.tile_pool(name="vpool", bufs=2))
        opool = ctx.enter_context(tc.tile_pool(name="opool", bufs=10))
        tmp = ctx.enter_context(tc.tile_pool(name="tmp", bufs=3))
        psum = ctx.enter_context(tc.tile_pool(name="psum", bufs=8, space="PSUM"))

        # ---- constants / small inputs ----
        # DMA queue split: qkvw + hT on Sync (HWDGE ring 1), K/V cache on
        # GpSimd (SWDGE), ow + bias on Scalar (HWDGE ring 2) — three
        # independent rings so the aggregate approaches the 435 GB/s
        # SBUF-fabric ceiling instead of one ~340 GB/s queue.
        hT_sb = consts.tile([128, 40 * B], bf16)         # (E%128, (Echunk, b))
        nc.sync.dma_start(out=hT_sb[:], in_=hT[:])
        bias_sb = consts.tile([128, B * HPC * NCH], f32)  # (t%128, (b, h, chunk))
        nc.scalar.dma_start(out=bias_sb[:], in_=bias[:])
        ones_col = consts.tile([128, 1], f32)
        nc.vector.memset(ones_col[:], 1.0)
        ones_row = consts.tile([1, 128], f32)
        nc.vector.memset(ones_row[:], 1.0)

        qT_sb = consts.tile([128, HPC * B], bf16)   # col = h*B + b ; partition = d
        kT_sb = consts.tile([128, HPC * B], bf16)
        vT_sb = consts.tile([128, HPC * B], bf16)
        colsum_sb = consts.tile([128, HPC * B], f32)
        aoT_sb = consts.tile([128, HPC * B], f32)   # unnormalized attn@V ^T
        out_sb = consts.tile([B, E], f32)

        # ---- fused QKV projection (weight-stationary, bf16 FWL) ----
        # psum[oc] [128, B] accumulated over 40 E-chunks;
        # lhsT = W chunk [128(E), 128(outcol)], rhs = hT chunk [128(E), B].
        for w in range(3):  # 0=q (pre-scaled on host), 1=k, 2=v
            dst = (qT_sb, kT_sb, vT_sb)[w]
            ps = [psum.tile([128, B], f32, tag="ps", name=f"ps_qkv{w}_{i}")
                  for i in range(HPC)]
            for g in range(5):  # groups of 8 E-chunks (1.31 MB DMA each)
                wt = wpool.tile([128, 8 * EPC], bf16, tag="w")
                nc.sync.dma_start(out=wt[:], in_=qkvw[w, g])
                for kl in range(8):
                    kc = g * 8 + kl
                    for oc in range(HPC):
                        nc.tensor.matmul(
                            ps[oc][:],
                            lhsT=wt[:, kl * EPC + oc * 128: kl * EPC + (oc + 1) * 128],
                            rhs=hT_sb[:, kc * B:(kc + 1) * B],
                            start=(kc == 0),
                            stop=(kc == 39),
                        )
            for oc in range(HPC):
                nc.scalar.copy(dst[:, oc * B:(oc + 1) * B], ps[oc][:])

        # ---- attention per b (K/V for all 5 heads in one DMA each) ----
        for b in range(B):
            n = nch[b]
            sd = n * 128
            Kt = kpool.tile([128, HPC, nmax * 128], bf16, tag="K")
            nc.gpsimd.dma_start(out=Kt[:, :, :sd], in_=kt[b, :, :, :sd])
            Vt = vpool.tile([128, HPC, nmax, D], bf16, tag="V")
            nc.gpsimd.dma_start(out=Vt[:, :, :n, :], in_=vt[b, :, :, :n, :])
            for h in range(HPC):
                col = h * B + b
                sc_ps = psum.tile([128, NCH], f32, tag="ps", name=f"sc_{b}_{h}")
                for c in range(n):
                    nc.tensor.matmul(
                        sc_ps[:, c:c + 1],
                        lhsT=Kt[:, h, c * 128:(c + 1) * 128],
                        rhs=qT_sb[:, col:col + 1],
                        start=True,
                        stop=True,
                    )
                s_sb = tmp.tile([128, NCH], f32, tag="s")
                nc.vector.tensor_add(
                    s_sb[:, :n],
                    sc_ps[:, :n],
                    bias_sb[:, (b * HPC + h) * NCH:(b * HPC + h) * NCH + n],
                )
                attn_sb = tmp.tile([128, NCH], bf16, tag="attn")
                nc.scalar.activation(
                    attn_sb[:, :n],
                    s_sb[:, :n],
                    func=mybir.ActivationFunctionType.Exp,
                    accum_out=colsum_sb[:, col:col + 1],
                )
                ao_ps = psum.tile([128, 1], f32, tag="ps", name=f"ao_{b}_{h}")
                for c in range(n):
                    nc.tensor.matmul(
                        ao_ps[:],
                        lhsT=Vt[:, h, c, :],
                        rhs=attn_sb[:, c:c + 1],
                        start=(c == 0),
                        stop=(c == n - 1),
                    )
                nc.scalar.copy(aoT_sb[:, col:col + 1], ao_ps[:])

        # ---- new-token term, batched over all 20 (b,h) ----
        # a_pos = exp(q . k_new) (alibi bias at own position is 0); the
        # stale cache column at pos was killed via bias[pos] = -1e30.
        qk_sb = tmp.tile([128, HPC * B], f32, tag="qk")
        nc.vector.tensor_mul(qk_sb[:], qT_sb[:], kT_sb[:])
        apos_ps = psum.tile([1, HPC * B], f32, tag="ps", name="apos_ps")
        nc.tensor.matmul(apos_ps[:], lhsT=ones_col[:], rhs=qk_sb[:],
                         start=True, stop=True)
        apos_sb = tmp.tile([1, HPC * B], f32, tag="apos")
        nc.scalar.activation(apos_sb[:], apos_ps[:],
                             func=mybir.ActivationFunctionType.Exp)

        # ---- softmax normalization (batched over all 20 (b,h)) ----
        sums_ps = psum.tile([1, HPC * B], f32, tag="ps", name="sums_ps")
        nc.tensor.matmul(sums_ps[:], lhsT=ones_col[:], rhs=colsum_sb[:],
                         start=True, stop=True)
        sums_sb = tmp.tile([1, HPC * B], f32, tag="sums")
        nc.vector.tensor_add(sums_sb[:], sums_ps[:], apos_sb[:])
        recip_sb = tmp.tile([1, HPC * B], f32, tag="recip")
        nc.vector.reciprocal(recip_sb[:], sums_sb[:])
        rb_ps = psum.tile([128, HPC * B], f32, tag="ps", name="rb_ps")
        nc.tensor.matmul(rb_ps[:], lhsT=ones_row[:], rhs=recip_sb[:],
                         start=True, stop=True)
        ap_ps = psum.tile([128, HPC * B], f32, tag="ps", name="ap_ps")
        nc.tensor.matmul(ap_ps[:], lhsT=ones_row[:], rhs=apos_sb[:],
                         start=True, stop=True)
        aon_sb = tmp.tile([128, HPC * B], f32, tag="aon")
        nc.vector.tensor_mul(aon_sb[:], ap_ps[:], vT_sb[:])
        aot_sb = tmp.tile([128, HPC * B], f32, tag="aot")
        nc.vector.tensor_add(aot_sb[:], aoT_sb[:], aon_sb[:])
        attn_nT = consts.tile([128, HPC * B], bf16)
        nc.vector.tensor_mul(attn_nT[:], aot_sb[:], rb_ps[:])

        # ---- output projection (natural): out[b, 5120] ----
        # lhsT = attn_nT slice [128(hd), B] stationary, rhs = o chunk
        # [128(hd), 512] moving, accumulate over the 5 head-chunks.
        for j in range(10):
            ot = opool.tile([128, HPC * 512], bf16, tag="ot")
            nc.scalar.dma_start(out=ot[:], in_=ow[j])
            ops = psum.tile([B, 512], f32, tag="ps", name=f"op_{j}")
            for h in range(HPC):
                nc.tensor.matmul(
                    ops[:],
                    lhsT=attn_nT[:, h * B:(h + 1) * B],
                    rhs=ot[:, h * 512:(h + 1) * 512],
                    start=(h == 0),
                    stop=(h == HPC - 1),
                )
            nc.scalar.copy(out_sb[:, j * 512:(j + 1) * 512], ops[:])

        nc.gpsimd.dma_start(out=out[:], in_=out_sb[:])

    nc.compile()  # Bacc finalize: splits multi-waits (matmul 1-wait limit)
    return nc


def _prepare_core_inputs(core, hidden, qkv_w, o_w, k_cache, v_cache, bt, sl, pos):
    hs = slice(core * HPC, (core + 1) * HPC)
    es = slice(core * EPC, (core + 1) * EPC)

    # qkvw: [3, 5(g), 128(p), 8(kl)*640(c)]; E index e = (g*8+kl)*128 + p
    qkvw = np.ascontiguousarray(qkv_w[:, :, es])
    qkvw[0] *= np.float32(D ** -0.5)
    qkvw_t = (
        qkvw.reshape(3, 5, 8, 128, EPC)
        .transpose(0, 1, 3, 2, 4)
        .reshape(3, 5, 128, 8 * EPC)
        .astype(BF16)
    )

    # ow: [10(j), 128(p), 5(h)*512(c')]; row hd = h*128 + p, col = j*512 + c'
    ow_t = (
        np.ascontiguousarray(o_w[es, :])
        .reshape(HPC, 128, 10, 512)
        .transpose(2, 1, 0, 3)
        .reshape(10, 128, HPC * 512)
        .astype(BF16)
    )

    kg = k_cache[:, hs]  # [NB, HPC, BS, D]
    vg = v_cache[:, hs]
    kt = np.empty((B, 128, HPC, S), BF16)     # [b, d, h, t]
    vt = np.empty((B, 128, HPC, NCH, D), BF16)  # [b, t%128, h, t//128, d]
    for b in range(B):
        kk = kg[bt[b]].transpose(1, 0, 2, 3).reshape(HPC, S, D)
        kt[b] = kk.transpose(2, 0, 1).astype(BF16)
        vv = vg[bt[b]].transpose(1, 0, 2, 3).reshape(HPC, NCH, 128, D)
        vt[b] = vv.transpose(2, 0, 1, 3).astype(BF16)

    slopes = _alibi_slopes(H)[core * HPC:(core + 1) * HPC]
    t_in = np.arange(128)[:, None]
    tg = (np.arange(NCH)[None, :] * 128 + t_in).astype(np.float32)  # [128, 16]
    bias = np.empty((128, B, HPC, NCH), np.float32)
    for b in range(B):
        for h in range(HPC):
            val = slopes[h] * (tg - np.float32(pos[b]))
            val[tg >= sl[b]] = NEG
            val[tg == pos[b]] = NEG  # stale cache col at pos: new-token term is separate
            bias[:, b, h, :] = val

    hTf = np.ascontiguousarray(
        hidden.T.reshape(40, 128, B).transpose(1, 0, 2).reshape(128, 40 * B)
    ).astype(BF16)

    return dict(
        hT=hTf,
        qkvw=qkvw_t,
        ow=ow_t,
        kt=kt,
        vt=vt,
        bias=np.ascontiguousarray(bias.reshape(128, B * HPC * NCH)),
    )


def kernel(**inputs):
    global LAST_RESULTS
    hidden = np.asarray(inputs["hidden_states"], np.float32)
    qkv_w = np.asarray(inputs["qkv_weight"], np.float32)
    o_w = np.asarray(inputs["o_proj_weight"], np.float32)
    k_cache = np.asarray(inputs["k_cache"], np.float32)
    v_cache = np.asarray(inputs["v_cache"], np.float32)
    bt = np.asarray(inputs["block_tables"]).astype(np.int64)
    sl = np.asarray(inputs["sequence_lengths"]).astype(np.int64)

    pos = tuple(int(x) - 1 for x in sl)
    nch = tuple(int(math.ceil(int(x) / 128)) for x in sl)

    in_maps = [
        _prepare_core_inputs(c, hidden, qkv_w, o_w, k_cache, v_cache, bt, sl, pos)
        for c in range(NCORES)
    ]

    if nch not in _PROGRAM_CACHE:
        _PROGRAM_CACHE[nch] = _build_program(nch)
    nc = _PROGRAM_CACHE[nch]

    from concourse.bass_utils import run_bass_kernel_spmd

    res = run_bass_kernel_spmd(
        nc,
        in_maps,
        core_ids=list(range(NCORES)),
        trace=bool(os.environ.get("BASS_TRACE")),
    )
    LAST_RESULTS = res

    out = np.zeros((B, E), np.float64)
    for c in range(NCORES):
        out += np.asarray(res.results[c]["out"]).astype(np.float64)
    return out.astype(np.float32)
